# revision 13
# baseline (speedup 1.0000x reference)
"""Trainium2 Bass kernel for nn_Decoder_12128987644664.

Pipeline (per input contract: takes FULL inputs, returns FULL output):
  1. Launch 1 (8 cores, data-parallel over batch): size_pred MLP
     (Linear -> LayerNorm -> ReLU -> Linear) on zT shards, emitting
     n_pred^T [17, 4096] per core.  Matmuls run in float32r.
  2. Host: argmax over n_pred (with exact float64 recompute of near-tie
     rows so the ragged sizes match a faithful fp32 reference bit-for-bit),
     ragged index construction, and a per-core sort of samples by
     descending n.
  3. Launch 2 (8 cores): the decoder MLP over the ragged rows.  Rows are
     grouped by within-sample position k; since
     (z_b * key_k) @ W1 == z_b @ (diag(key_k) @ W1), each group k is a
     dense matmul over a prefix of the sorted z shard (kept resident in
     SBUF), with the key folded into W1 on the scalar engine.  No gather,
     no HBM read amplification.
  4. Host: scatter rows back to the reference ragged order.

The key_net is a 17-row table (one-hot @ W is a row lookup) computed on
host in float64.
"""

import os
import numpy as np

import concourse.bass as bass
import concourse.bacc as bacc
import concourse.tile as tile
from concourse import mybir
from concourse import bass_utils

F32 = mybir.dt.float32
F32R = mybir.dt.float32r
AF = mybir.ActivationFunctionType

NCORES = 8
B = 32768
BPC = B // NCORES          # 4096 batch rows per core
HID = 512
DIM = 256
MAX_N = 17
SP_MID = (HID + MAX_N) // 2   # 264
DEC_MID = (HID + DIM) // 2    # 384
EPS = 1e-5
CHUNK = 512                # moving-dim chunk (fp32 moving max)

# partition chunking helpers: list of (offset, size) covering `total`
def _pchunks(total):
    out = []
    off = 0
    while off < total:
        sz = min(128, total - off)
        out.append((off, sz))
        off += sz
    return out

SP_CH = _pchunks(SP_MID)    # [(0,128),(128,128),(256,8)]
DEC_CH = _pchunks(DEC_MID)  # 3 x 128
DIM_CH = _pchunks(DIM)      # 2 x 128
HID_CH = _pchunks(HID)      # 4 x 128

_compiled = {}
DIAG = {}


# ---------------------------------------------------------------- launch 1
def _build_sizepred():
    key = ("sizepred", BPC)
    if key in _compiled:
        return _compiled[key]
    nc = bacc.Bacc("TRN2", target_bir_lowering=False, debug=False,
                   num_devices=NCORES)
    zt_d = nc.dram_tensor("zt", [HID, BPC], F32R, kind="ExternalInput").ap()
    w1_d = nc.dram_tensor("spw1", [HID, SP_MID], F32R, kind="ExternalInput").ap()
    b1_d = nc.dram_tensor("spb1", [SP_MID, 1], F32, kind="ExternalInput").ap()
    g1_d = nc.dram_tensor("spg1", [SP_MID, 1], F32, kind="ExternalInput").ap()
    be1_d = nc.dram_tensor("spbe1", [SP_MID, 1], F32, kind="ExternalInput").ap()
    w2_d = nc.dram_tensor("spw2", [SP_MID, MAX_N], F32R, kind="ExternalInput").ap()
    b2_d = nc.dram_tensor("spb2", [MAX_N, 1], F32, kind="ExternalInput").ap()
    onesm_d = nc.dram_tensor("onesm", [128, 1], F32R, kind="ExternalInput").ap()
    ones1_d = nc.dram_tensor("ones1", [1, 128], F32R, kind="ExternalInput").ap()
    np_d = nc.dram_tensor("npredT", [MAX_N, BPC], F32, kind="ExternalOutput").ap()

    with tile.TileContext(nc) as tc:
        with tc.tile_pool(name="const", bufs=1) as cpool, \
             tc.tile_pool(name="zts", bufs=1) as zpool, \
             tc.tile_pool(name="work", bufs=2) as wpool, \
             tc.tile_pool(name="small", bufs=2) as spool, \
             tc.tile_pool(name="ps_h", bufs=1, space="PSUM") as ps_h, \
             tc.tile_pool(name="ps_s", bufs=1, space="PSUM") as ps_s, \
             tc.tile_pool(name="ps_b", bufs=1, space="PSUM") as ps_b, \
             tc.tile_pool(name="ps_n", bufs=1, space="PSUM") as ps_n:

            # constants
            w1_sb = []
            for i, (o, s) in enumerate(HID_CH):
                t = cpool.tile([s, SP_MID], F32R, tag=f"w1_{i}")
                nc.sync.dma_start(t[:], w1_d[o:o + s, :])
                w1_sb.append(t)
            w2_sb, b1_sb, g1_sb, be1_sb = [], [], [], []
            for m, (o, s) in enumerate(SP_CH):
                t = cpool.tile([s, MAX_N], F32R, tag=f"w2_{m}")
                nc.sync.dma_start(t[:], w2_d[o:o + s, :])
                w2_sb.append(t)
                for lst, src, nm in ((b1_sb, b1_d, "b1"), (g1_sb, g1_d, "g1"),
                                     (be1_sb, be1_d, "be1")):
                    tt = cpool.tile([s, 1], F32, tag=f"{nm}_{m}")
                    nc.sync.dma_start(tt[:], src[o:o + s, :])
                    lst.append(tt)
            b2_sb = cpool.tile([MAX_N, 1], F32, tag="b2")
            nc.sync.dma_start(b2_sb[:], b2_d[:])
            # ones columns scaled by 1/SP_MID for mean via matmul
            onesm = cpool.tile([128, 1], F32R, tag="onesm")
            nc.sync.dma_start(onesm[:], onesm_d[:])
            # ones row for partition-broadcast (K=1 matmul)
            ones1 = cpool.tile([1, 128], F32R, tag="ones1")
            nc.sync.dma_start(ones1[:], ones1_d[:])
            epsb = cpool.tile([1, 1], F32, tag="epsb")
            nc.vector.memset(epsb[:], EPS)

            # z^T resident, DMA'd in column chunks
            zt_sb = []
            for i, (o, s) in enumerate(HID_CH):
                zt_sb.append(zpool.tile([s, BPC], F32R, tag=f"zt{i}",
                                        name=f"zt{i}"))
            nchunks = (BPC + CHUNK - 1) // CHUNK
            for c in range(nchunks):
                c0 = c * CHUNK
                ncs = min(CHUNK, BPC - c0)
                for i, (o, s) in enumerate(HID_CH):
                    nc.sync.dma_start(zt_sb[i][:, c0:c0 + ncs],
                                      zt_d[o:o + s, c0:c0 + ncs])

            for c in range(nchunks):
                c0 = c * CHUNK
                ncs = min(CHUNK, BPC - c0)
                # ---- mm1: hpre^T = spW1^T @ z^T + b1
                hpre = []
                for m, (mo, ms) in enumerate(SP_CH):
                    ph = ps_h.tile([ms, ncs], F32, tag=f"ph{m}")
                    for i, (io, isz) in enumerate(HID_CH):
                        nc.tensor.matmul(
                            ph[:], lhsT=w1_sb[i][:, mo:mo + ms],
                            rhs=zt_sb[i][:, c0:c0 + ncs],
                            start=(i == 0), stop=(i == len(HID_CH) - 1))
                    h = wpool.tile([ms, ncs], F32R, tag=f"hpre{m}")
                    nc.vector.tensor_scalar_add(h[:], ph[:], b1_sb[m][:])
                    hpre.append(h)
                # ---- LN stats via ones-matmul (cross-partition sums)
                pmean = ps_s.tile([1, ncs], F32, tag="sum")
                for m, (mo, ms) in enumerate(SP_CH):
                    nc.tensor.matmul(pmean[:], lhsT=onesm[0:ms, :],
                                     rhs=hpre[m][:],
                                     start=(m == 0), stop=(m == len(SP_CH) - 1))
                hsq = []
                for m, (mo, ms) in enumerate(SP_CH):
                    t = wpool.tile([ms, ncs], F32R, tag=f"hsq{m}")
                    nc.gpsimd.tensor_mul(t[:], hpre[m][:], hpre[m][:])
                    hsq.append(t)
                pmsq = ps_s.tile([1, ncs], F32, tag="ssq")
                for m, (mo, ms) in enumerate(SP_CH):
                    nc.tensor.matmul(pmsq[:], lhsT=onesm[0:ms, :],
                                     rhs=hsq[m][:],
                                     start=(m == 0), stop=(m == len(SP_CH) - 1))
                # var = E[x^2] - mean^2 ; rstd = 1/sqrt(var+eps)
                msq = spool.tile([1, ncs], F32, tag="msq")
                nc.scalar.activation(msq[:], pmean[:], AF.Square)
                var = spool.tile([1, ncs], F32, tag="var")
                nc.vector.tensor_sub(var[:], pmsq[:], msq[:])
                rstd_r = spool.tile([1, ncs], F32R, tag="rstd_r")
                nc.scalar.activation(rstd_r[:], var[:], AF.Abs_reciprocal_sqrt,
                                     bias=epsb[:])
                pm_r = spool.tile([1, ncs], F32R, tag="pm_r")
                nc.scalar.activation(pm_r[:], pmean[:], AF.Identity)
                # broadcast across partitions via K=1 matmul
                br = ps_b.tile([128, ncs], F32, tag="br")
                nc.tensor.matmul(br[:], lhsT=ones1[:], rhs=rstd_r[:],
                                 start=True, stop=True)
                bm = ps_b.tile([128, ncs], F32, tag="bm")
                nc.tensor.matmul(bm[:], lhsT=ones1[:], rhs=pm_r[:],
                                 start=True, stop=True)
                # ---- normalize + affine + relu
                nh = []
                for m, (mo, ms) in enumerate(SP_CH):
                    t1 = wpool.tile([ms, ncs], F32, tag=f"t1{m}")
                    nc.vector.tensor_sub(t1[:], hpre[m][:], bm[0:ms, :])
                    t2 = wpool.tile([ms, ncs], F32, tag=f"t2{m}")
                    nc.vector.scalar_tensor_tensor(
                        t2[:], t1[:], g1_sb[m][:], br[0:ms, :],
                        op0=mybir.AluOpType.mult, op1=mybir.AluOpType.mult)
                    t3 = wpool.tile([ms, ncs], F32R, tag=f"nh{m}")
                    nc.scalar.activation(t3[:], t2[:], AF.Relu,
                                         bias=be1_sb[m][:])
                    nh.append(t3)
                # ---- mm2: n_pred^T = spW2^T @ nh + b2
                pnp = ps_n.tile([MAX_N, ncs], F32, tag="np")
                for m, (mo, ms) in enumerate(SP_CH):
                    nc.tensor.matmul(pnp[:], lhsT=w2_sb[m][:],
                                     rhs=nh[m][:],
                                     start=(m == 0), stop=(m == len(SP_CH) - 1))
                npo = wpool.tile([MAX_N, ncs], F32, tag="npo")
                nc.scalar.activation(npo[:], pnp[:], AF.Identity,
                                     bias=b2_sb[:])
                nc.sync.dma_start(np_d[:, c0:c0 + ncs], npo[:])

    nc.compile()
    _compiled[key] = nc
    return nc


# ---------------------------------------------------------------- launch 2
def _build_decoder(mlist, tpad):
    key = ("decoder", tuple(mlist), tpad, BPC)
    if key in _compiled:
        return _compiled[key]
    nc = bacc.Bacc("TRN2", target_bir_lowering=False, debug=False,
                   num_devices=NCORES)
    zt_d = nc.dram_tensor("zts", [HID, BPC], F32R, kind="ExternalInput").ap()
    w1_d = nc.dram_tensor("dw1", [HID, DEC_MID], F32, kind="ExternalInput").ap()
    b1_d = nc.dram_tensor("db1", [DEC_MID, 1], F32, kind="ExternalInput").ap()
    w2_d = nc.dram_tensor("dw2", [DEC_MID, DIM], F32R, kind="ExternalInput").ap()
    b2_d = nc.dram_tensor("db2", [DIM, 1], F32, kind="ExternalInput").ap()
    kt_d = nc.dram_tensor("keysT", [HID, MAX_N], F32, kind="ExternalInput").ap()
    xt_d = nc.dram_tensor("xT", [DIM, tpad], F32, kind="ExternalOutput").ap()

    with tile.TileContext(nc) as tc:
        with tc.tile_pool(name="const", bufs=1) as cpool, \
             tc.tile_pool(name="zts", bufs=1) as zpool, \
             tc.tile_pool(name="w1k", bufs=2) as kpool, \
             tc.tile_pool(name="hber", bufs=2) as hpool, \
             tc.tile_pool(name="xout", bufs=2) as xpool, \
             tc.tile_pool(name="ps_h", bufs=2, space="PSUM") as ps_h, \
             tc.tile_pool(name="ps_x", bufs=1, space="PSUM") as ps_x:

            w1_sb, kt_sb = [], []
            for i, (o, s) in enumerate(HID_CH):
                t = cpool.tile([s, DEC_MID], F32, tag=f"w1_{i}")
                nc.sync.dma_start(t[:], w1_d[o:o + s, :])
                w1_sb.append(t)
                t = cpool.tile([s, MAX_N], F32, tag=f"kt_{i}")
                nc.sync.dma_start(t[:], kt_d[o:o + s, :])
                kt_sb.append(t)
            w2_sb, b1_sb = [], []
            for m, (o, s) in enumerate(DEC_CH):
                t = cpool.tile([s, DIM], F32R, tag=f"w2_{m}")
                nc.sync.dma_start(t[:], w2_d[o:o + s, :])
                w2_sb.append(t)
                tt = cpool.tile([s, 1], F32, tag=f"b1_{m}")
                nc.sync.dma_start(tt[:], b1_d[o:o + s, :])
                b1_sb.append(tt)
            b2_sb = []
            for j, (o, s) in enumerate(DIM_CH):
                tt = cpool.tile([s, 1], F32, tag=f"b2_{j}")
                nc.sync.dma_start(tt[:], b2_d[o:o + s, :])
                b2_sb.append(tt)

            # sorted z^T resident in SBUF; column-chunked DMA
            zt_sb = []
            for i, (o, s) in enumerate(HID_CH):
                zt_sb.append(zpool.tile([s, BPC], F32R, tag=f"zt{i}",
                                        name=f"zt{i}"))
            nchunks = (BPC + CHUNK - 1) // CHUNK
            for c in range(nchunks):
                c0 = c * CHUNK
                ncs = min(CHUNK, BPC - c0)
                for i, (o, s) in enumerate(HID_CH):
                    nc.sync.dma_start(zt_sb[i][:, c0:c0 + ncs],
                                      zt_d[o:o + s, c0:c0 + ncs])

            goff = 0
            for k in range(MAX_N - 1):
                mk = mlist[k]
                if mk == 0:
                    continue
                # W1k = diag(key_k) @ W1 on gpsimd (idle engine; converts to f32r)
                w1k = []
                for i in range(len(HID_CH)):
                    t = kpool.tile([HID_CH[i][1], DEC_MID], F32R,
                                   tag=f"w1k{i}", name=f"w1k{i}")
                    nc.gpsimd.tensor_scalar_mul(t[:], w1_sb[i][:],
                                                kt_sb[i][:, k:k + 1])
                    w1k.append(t)
                for c0 in range(0, mk, CHUNK):
                    ncs = min(CHUNK, mk - c0)
                    hts = []
                    for m, (mo, ms) in enumerate(DEC_CH):
                        ph = ps_h.tile([ms, ncs], F32, tag=f"ph{m}")
                        for i in range(len(HID_CH)):
                            nc.tensor.matmul(
                                ph[:], lhsT=w1k[i][:, mo:mo + ms],
                                rhs=zt_sb[i][:, c0:c0 + ncs],
                                start=(i == 0), stop=(i == len(HID_CH) - 1))
                        ht = hpool.tile([ms, ncs], F32R, tag=f"h{m}")
                        nc.scalar.activation(ht[:], ph[:], AF.Relu,
                                             bias=b1_sb[m][:])
                        hts.append(ht)
                    for j, (jo, js) in enumerate(DIM_CH):
                        px = ps_x.tile([js, ncs], F32, tag=f"px{j}")
                        for m in range(len(DEC_CH)):
                            nc.tensor.matmul(
                                px[:], lhsT=w2_sb[m][:, jo:jo + js],
                                rhs=hts[m][:],
                                start=(m == 0), stop=(m == len(DEC_CH) - 1))
                        xs = xpool.tile([js, ncs], F32, tag=f"xs{j}")
                        nc.vector.tensor_scalar_add(xs[:], px[:], b2_sb[j][:])
                        nc.sync.dma_start(
                            xt_d[jo:jo + js, goff + c0:goff + c0 + ncs], xs[:])
                goff += mk

    nc.compile()
    _compiled[key] = nc
    return nc


# ---------------------------------------------------------------- host math
def _keys_table(key_W1, key_b1, key_g1, key_be1, key_W2, key_b2):
    """The key_net on the 17 one-hot rows, in float64."""
    pre = key_W1.astype(np.float64) + key_b1.astype(np.float64)
    m = pre.mean(-1, keepdims=True)
    v = pre.var(-1, keepdims=True)
    ln = (pre - m) / np.sqrt(v + EPS) * key_g1.astype(np.float64) \
        + key_be1.astype(np.float64)
    keys = np.maximum(ln, 0.0) @ key_W2.astype(np.float64) \
        + key_b2.astype(np.float64)
    return keys  # [17, 512] float64


def _sizepred_exact(z_rows, sp_W1, sp_b1, sp_g1, sp_be1, sp_W2, sp_b2):
    """float64 replica of the reference size_pred MLP for selected rows."""
    h = z_rows.astype(np.float64) @ sp_W1.astype(np.float64) \
        + sp_b1.astype(np.float64)
    m = h.mean(-1, keepdims=True)
    v = h.var(-1, keepdims=True)
    ln = (h - m) / np.sqrt(v + EPS) * sp_g1.astype(np.float64) \
        + sp_be1.astype(np.float64)
    ln = np.maximum(ln, 0.0)
    return ln @ sp_W2.astype(np.float64) + sp_b2.astype(np.float64)


MARGIN = 0.05  # near-tie threshold for exact host recompute of argmax rows


def kernel(z, key_W1, key_b1, key_g1, key_be1, key_W2, key_b2,
           dec_W1, dec_b1, dec_W2, dec_b2,
           sp_W1, sp_b1, sp_g1, sp_be1, sp_W2, sp_b2):
    z = np.asarray(z, dtype=np.float32)
    to32 = lambda a: np.ascontiguousarray(np.asarray(a), dtype=np.float32)
    key_W1, key_b1, key_g1, key_be1, key_W2, key_b2 = map(
        to32, (key_W1, key_b1, key_g1, key_be1, key_W2, key_b2))
    dec_W1, dec_b1, dec_W2, dec_b2 = map(to32, (dec_W1, dec_b1, dec_W2, dec_b2))
    sp_W1, sp_b1, sp_g1, sp_be1, sp_W2, sp_b2 = map(
        to32, (sp_W1, sp_b1, sp_g1, sp_be1, sp_W2, sp_b2))

    col = lambda a: np.ascontiguousarray(a.reshape(-1, 1))

    # ---------------- launch 1: size_pred
    nc1 = _build_sizepred()
    zsh = z.reshape(NCORES, BPC, HID)
    in_maps = []
    for c in range(NCORES):
        zt = np.ascontiguousarray(zsh[c].T)  # [512, 4096]
        in_maps.append(dict(zt=zt, spw1=sp_W1, spb1=col(sp_b1),
                            spg1=col(sp_g1), spbe1=col(sp_be1),
                            spw2=sp_W2, spb2=col(sp_b2),
                            onesm=np.full((128, 1), 1.0 / SP_MID, np.float32),
                            ones1=np.ones((1, 128), np.float32)))
    res1 = bass_utils.run_bass_kernel_spmd(nc1, in_maps,
                                           core_ids=list(range(NCORES)))
    DIAG["res1"] = res1
    n_pred = np.concatenate(
        [res1.results[c]["npredT"].T for c in range(NCORES)], axis=0)
    DIAG["n_pred_dev"] = n_pred

    # ---------------- argmax with exact near-tie patch
    n = n_pred.argmax(-1).astype(np.int64)
    part = np.partition(n_pred, MAX_N - 2, axis=-1)
    margin = part[:, -1] - part[:, -2]
    risky = np.flatnonzero(margin < MARGIN)
    DIAG["n_risky"] = len(risky)
    if len(risky):
        np_exact = _sizepred_exact(z[risky], sp_W1, sp_b1, sp_g1, sp_be1,
                                   sp_W2, sp_b2)
        n[risky] = np_exact.argmax(-1)

    # ---------------- ragged structure
    T = int(n.sum())
    batch = np.repeat(np.arange(B, dtype=np.int64), n).astype(np.int32)
    starts = np.zeros(B, dtype=np.int64)
    np.cumsum(n[:-1], out=starts[1:])

    keys64 = _keys_table(key_W1, key_b1, key_g1, key_be1, key_W2, key_b2)
    keysT = np.ascontiguousarray(keys64.T.astype(np.float32))  # [512, 17]

    # per-core sort by descending n
    nsh = n.reshape(NCORES, BPC)
    orders = [np.argsort(-nsh[c], kind="stable") for c in range(NCORES)]
    mks = np.stack([(nsh[c][:, None] > np.arange(MAX_N - 1)[None, :]).sum(0)
                    for c in range(NCORES)])          # [8, 16]
    # common padded sizes; even-padded (fp32r matmul dst width must be even)
    mlist = [min(BPC, int(m + (m & 1))) for m in mks.max(0).astype(int)]
    tpad = int(sum(mlist))

    # ---------------- launch 2: decoder
    nc2 = _build_decoder(mlist, tpad)
    in_maps = []
    for c in range(NCORES):
        zs = zsh[c][orders[c]]                        # sorted rows [4096, 512]
        zts = np.ascontiguousarray(zs.T)              # [512, 4096]
        in_maps.append(dict(zts=zts, dw1=dec_W1, db1=col(dec_b1),
                            dw2=dec_W2, db2=col(dec_b2), keysT=keysT))
    res2 = bass_utils.run_bass_kernel_spmd(nc2, in_maps,
                                           core_ids=list(range(NCORES)))
    DIAG["res2"] = res2

    # ---------------- host scatter back to ragged order
    offs = np.concatenate([[0], np.cumsum(mlist)]).astype(np.int64)
    x = np.empty((T, DIM), dtype=np.float32)
    core_T = nsh.sum(1)
    base = np.concatenate([[0], np.cumsum(core_T)]).astype(np.int64)
    for c in range(NCORES):
        xt = res2.results[c]["xT"]                    # [256, tpad]
        starts_c = starts[c * BPC:(c + 1) * BPC] - base[c]
        src_cols, dst_rows = [], []
        for k in range(MAX_N - 1):
            mk = int(mks[c, k])
            if mk == 0:
                continue
            o = orders[c][:mk]
            dst_rows.append(base[c] + starts_c[o] + k)
            src_cols.append(offs[k] + np.arange(mk))
        dst = np.concatenate(dst_rows)
        src = np.concatenate(src_cols)
        x[dst] = np.ascontiguousarray(xt.T)[src]
    return x, batch


# revision 17
# speedup vs baseline: 1.2991x; 1.2991x over previous
"""Trainium2 Bass kernel for nn_Decoder_12128987644664.

Pipeline (per input contract: takes FULL inputs, returns FULL output):
  1. Launch 1 (8 cores, data-parallel over batch): size_pred MLP
     (Linear -> LayerNorm -> ReLU -> Linear) on zT shards, emitting
     n_pred^T [17, 4096] per core.  Matmuls run in float32r.
  2. Host: argmax over n_pred (with exact float64 recompute of near-tie
     rows so the ragged sizes match a faithful fp32 reference bit-for-bit),
     ragged index construction, and a per-core sort of samples by
     descending n.
  3. Launch 2 (8 cores): the decoder MLP over the ragged rows.  Rows are
     grouped by within-sample position k; since
     (z_b * key_k) @ W1 == z_b @ (diag(key_k) @ W1), each group k is a
     dense matmul over a prefix of the sorted z shard (kept resident in
     SBUF), with the key folded into W1 on the scalar engine.  No gather,
     no HBM read amplification.
  4. Host: scatter rows back to the reference ragged order.

The key_net is a 17-row table (one-hot @ W is a row lookup) computed on
host in float64.
"""

import os
import numpy as np

import concourse.bass as bass
import concourse.bacc as bacc
import concourse.tile as tile
from concourse import mybir
from concourse import bass_utils

F32 = mybir.dt.float32
F32R = mybir.dt.float32r
AF = mybir.ActivationFunctionType

NCORES = 8
B = 32768
BPC = B // NCORES          # 4096 batch rows per core
HID = 512
DIM = 256
MAX_N = 17
SP_MID = (HID + MAX_N) // 2   # 264
DEC_MID = (HID + DIM) // 2    # 384
EPS = 1e-5
CHUNK = 512                # moving-dim chunk (fp32 moving max)

# partition chunking helpers: list of (offset, size) covering `total`
def _pchunks(total):
    out = []
    off = 0
    while off < total:
        sz = min(128, total - off)
        out.append((off, sz))
        off += sz
    return out

SP_CH = _pchunks(SP_MID)    # [(0,128),(128,128),(256,8)]
DEC_CH = _pchunks(DEC_MID)  # 3 x 128
DIM_CH = _pchunks(DIM)      # 2 x 128
HID_CH = _pchunks(HID)      # 4 x 128

_compiled = {}
DIAG = {}


# ---------------------------------------------------------------- launch 1
def _build_sizepred():
    key = ("sizepred", BPC)
    if key in _compiled:
        return _compiled[key]
    nc = bacc.Bacc("TRN2", target_bir_lowering=False, debug=False,
                   num_devices=NCORES)
    zt_d = nc.dram_tensor("zt", [HID, BPC], F32R, kind="ExternalInput").ap()
    w1_d = nc.dram_tensor("spw1", [HID, SP_MID], F32R, kind="ExternalInput").ap()
    b1_d = nc.dram_tensor("spb1", [SP_MID, 1], F32, kind="ExternalInput").ap()
    g1_d = nc.dram_tensor("spg1", [SP_MID, 1], F32, kind="ExternalInput").ap()
    be1_d = nc.dram_tensor("spbe1", [SP_MID, 1], F32, kind="ExternalInput").ap()
    w2_d = nc.dram_tensor("spw2", [SP_MID, MAX_N], F32R, kind="ExternalInput").ap()
    b2_d = nc.dram_tensor("spb2", [MAX_N, 1], F32, kind="ExternalInput").ap()
    onesm_d = nc.dram_tensor("onesm", [128, 1], F32R, kind="ExternalInput").ap()
    ones1_d = nc.dram_tensor("ones1", [1, 128], F32R, kind="ExternalInput").ap()
    np_d = nc.dram_tensor("npredT", [MAX_N, BPC], F32, kind="ExternalOutput").ap()

    with tile.TileContext(nc) as tc:
        with tc.tile_pool(name="const", bufs=1) as cpool, \
             tc.tile_pool(name="zts", bufs=1) as zpool, \
             tc.tile_pool(name="work", bufs=2) as wpool, \
             tc.tile_pool(name="small", bufs=2) as spool, \
             tc.tile_pool(name="ps_h", bufs=1, space="PSUM") as ps_h, \
             tc.tile_pool(name="ps_s", bufs=1, space="PSUM") as ps_s, \
             tc.tile_pool(name="ps_b", bufs=1, space="PSUM") as ps_b, \
             tc.tile_pool(name="ps_n", bufs=1, space="PSUM") as ps_n:

            # constants
            w1_sb = []
            for i, (o, s) in enumerate(HID_CH):
                t = cpool.tile([s, SP_MID], F32R, tag=f"w1_{i}")
                nc.sync.dma_start(t[:], w1_d[o:o + s, :])
                w1_sb.append(t)
            w2_sb, b1_sb, g1_sb, be1_sb = [], [], [], []
            for m, (o, s) in enumerate(SP_CH):
                t = cpool.tile([s, MAX_N], F32R, tag=f"w2_{m}")
                nc.sync.dma_start(t[:], w2_d[o:o + s, :])
                w2_sb.append(t)
                for lst, src, nm in ((b1_sb, b1_d, "b1"), (g1_sb, g1_d, "g1"),
                                     (be1_sb, be1_d, "be1")):
                    tt = cpool.tile([s, 1], F32, tag=f"{nm}_{m}")
                    nc.sync.dma_start(tt[:], src[o:o + s, :])
                    lst.append(tt)
            b2_sb = cpool.tile([MAX_N, 1], F32, tag="b2")
            nc.sync.dma_start(b2_sb[:], b2_d[:])
            # ones columns scaled by 1/SP_MID for mean via matmul
            onesm = cpool.tile([128, 1], F32R, tag="onesm")
            nc.sync.dma_start(onesm[:], onesm_d[:])
            # ones row for partition-broadcast (K=1 matmul)
            ones1 = cpool.tile([1, 128], F32R, tag="ones1")
            nc.sync.dma_start(ones1[:], ones1_d[:])
            epsb = cpool.tile([1, 1], F32, tag="epsb")
            nc.vector.memset(epsb[:], EPS)

            # z^T resident, DMA'd in column chunks
            zt_sb = []
            for i, (o, s) in enumerate(HID_CH):
                zt_sb.append(zpool.tile([s, BPC], F32R, tag=f"zt{i}",
                                        name=f"zt{i}"))
            nchunks = (BPC + CHUNK - 1) // CHUNK
            for c in range(nchunks):
                c0 = c * CHUNK
                ncs = min(CHUNK, BPC - c0)
                for i, (o, s) in enumerate(HID_CH):
                    nc.sync.dma_start(zt_sb[i][:, c0:c0 + ncs],
                                      zt_d[o:o + s, c0:c0 + ncs])

            for c in range(nchunks):
                c0 = c * CHUNK
                ncs = min(CHUNK, BPC - c0)
                # ---- mm1: hpre^T = spW1^T @ z^T + b1
                hpre = []
                for m, (mo, ms) in enumerate(SP_CH):
                    ph = ps_h.tile([ms, ncs], F32, tag=f"ph{m}")
                    for i, (io, isz) in enumerate(HID_CH):
                        nc.tensor.matmul(
                            ph[:], lhsT=w1_sb[i][:, mo:mo + ms],
                            rhs=zt_sb[i][:, c0:c0 + ncs],
                            start=(i == 0), stop=(i == len(HID_CH) - 1))
                    h = wpool.tile([ms, ncs], F32R, tag=f"hpre{m}")
                    nc.scalar.activation(h[:], ph[:], AF.Identity,
                                         bias=b1_sb[m][:])
                    hpre.append(h)
                # ---- LN stats via ones-matmul (cross-partition sums)
                pmean = ps_s.tile([1, ncs], F32, tag="sum")
                for m, (mo, ms) in enumerate(SP_CH):
                    nc.tensor.matmul(pmean[:], lhsT=onesm[0:ms, :],
                                     rhs=hpre[m][:],
                                     start=(m == 0), stop=(m == len(SP_CH) - 1))
                hsq = []
                for m, (mo, ms) in enumerate(SP_CH):
                    t = wpool.tile([ms, ncs], F32R, tag=f"hsq{m}")
                    nc.gpsimd.tensor_mul(t[:], hpre[m][:], hpre[m][:])
                    hsq.append(t)
                pmsq = ps_s.tile([1, ncs], F32, tag="ssq")
                for m, (mo, ms) in enumerate(SP_CH):
                    nc.tensor.matmul(pmsq[:], lhsT=onesm[0:ms, :],
                                     rhs=hsq[m][:],
                                     start=(m == 0), stop=(m == len(SP_CH) - 1))
                # var = E[x^2] - mean^2 ; rstd = 1/sqrt(var+eps)
                msq = spool.tile([1, ncs], F32, tag="msq")
                nc.scalar.activation(msq[:], pmean[:], AF.Square)
                var = spool.tile([1, ncs], F32, tag="var")
                nc.vector.tensor_sub(var[:], pmsq[:], msq[:])
                rstd_r = spool.tile([1, ncs], F32R, tag="rstd_r")
                nc.scalar.activation(rstd_r[:], var[:], AF.Abs_reciprocal_sqrt,
                                     bias=epsb[:])
                mrs_r = spool.tile([1, ncs], F32R, tag="mrs_r")
                nc.vector.tensor_mul(mrs_r[:], pmean[:], rstd_r[:])
                # broadcast across partitions via K=1 matmul
                br = ps_b.tile([128, ncs], F32, tag="br")
                nc.tensor.matmul(br[:], lhsT=ones1[:], rhs=rstd_r[:],
                                 start=True, stop=True)
                bm = ps_b.tile([128, ncs], F32, tag="bm")
                nc.tensor.matmul(bm[:], lhsT=ones1[:], rhs=mrs_r[:],
                                 start=True, stop=True)
                # ---- normalize + affine + relu: relu((h*br - bm)*g + be)
                nh = []
                for m, (mo, ms) in enumerate(SP_CH):
                    t1 = wpool.tile([ms, ncs], F32, tag=f"t1{m}")
                    nc.vector.tensor_mul(t1[:], hpre[m][:], br[0:ms, :])
                    t2 = wpool.tile([ms, ncs], F32, tag=f"t2{m}")
                    nc.vector.tensor_sub(t2[:], t1[:], bm[0:ms, :])
                    t3 = wpool.tile([ms, ncs], F32R, tag=f"nh{m}")
                    nc.scalar.activation(t3[:], t2[:], AF.Relu,
                                         bias=be1_sb[m][:], scale=g1_sb[m][:])
                    nh.append(t3)
                # ---- mm2: n_pred^T = spW2^T @ nh + b2
                pnp = ps_n.tile([MAX_N, ncs], F32, tag="np")
                for m, (mo, ms) in enumerate(SP_CH):
                    nc.tensor.matmul(pnp[:], lhsT=w2_sb[m][:],
                                     rhs=nh[m][:],
                                     start=(m == 0), stop=(m == len(SP_CH) - 1))
                npo = wpool.tile([MAX_N, ncs], F32, tag="npo")
                nc.scalar.activation(npo[:], pnp[:], AF.Identity,
                                     bias=b2_sb[:])
                nc.sync.dma_start(np_d[:, c0:c0 + ncs], npo[:])

    nc.compile()
    _compiled[key] = nc
    return nc


# ---------------------------------------------------------------- launch 2
def _build_decoder(mlist, tpad):
    key = ("decoder", tuple(mlist), tpad, BPC)
    if key in _compiled:
        return _compiled[key]
    nc = bacc.Bacc("TRN2", target_bir_lowering=False, debug=False,
                   num_devices=NCORES)
    zt_d = nc.dram_tensor("zts", [HID, BPC], F32R, kind="ExternalInput").ap()
    w1_d = nc.dram_tensor("dw1", [HID, DEC_MID], F32, kind="ExternalInput").ap()
    b1_d = nc.dram_tensor("db1", [DEC_MID, 1], F32, kind="ExternalInput").ap()
    w2_d = nc.dram_tensor("dw2", [DEC_MID, DIM], F32R, kind="ExternalInput").ap()
    b2_d = nc.dram_tensor("db2", [DIM, 1], F32, kind="ExternalInput").ap()
    kt_d = nc.dram_tensor("keysT", [HID, MAX_N], F32, kind="ExternalInput").ap()
    xt_d = nc.dram_tensor("xT", [DIM, tpad], F32, kind="ExternalOutput").ap()

    with tile.TileContext(nc) as tc:
        with tc.tile_pool(name="const", bufs=1) as cpool, \
             tc.tile_pool(name="zts", bufs=1) as zpool, \
             tc.tile_pool(name="w1k", bufs=2) as kpool, \
             tc.tile_pool(name="hber", bufs=2) as hpool, \
             tc.tile_pool(name="xout", bufs=2) as xpool, \
             tc.tile_pool(name="ps_h", bufs=2, space="PSUM") as ps_h, \
             tc.tile_pool(name="ps_x", bufs=1, space="PSUM") as ps_x:

            w1_sb, kt_sb = [], []
            for i, (o, s) in enumerate(HID_CH):
                t = cpool.tile([s, DEC_MID], F32, tag=f"w1_{i}")
                nc.sync.dma_start(t[:], w1_d[o:o + s, :])
                w1_sb.append(t)
                t = cpool.tile([s, MAX_N], F32, tag=f"kt_{i}")
                nc.sync.dma_start(t[:], kt_d[o:o + s, :])
                kt_sb.append(t)
            w2_sb, b1_sb = [], []
            for m, (o, s) in enumerate(DEC_CH):
                t = cpool.tile([s, DIM], F32R, tag=f"w2_{m}")
                nc.sync.dma_start(t[:], w2_d[o:o + s, :])
                w2_sb.append(t)
                tt = cpool.tile([s, 1], F32, tag=f"b1_{m}")
                nc.sync.dma_start(tt[:], b1_d[o:o + s, :])
                b1_sb.append(tt)
            b2_sb = []
            for j, (o, s) in enumerate(DIM_CH):
                tt = cpool.tile([s, 1], F32, tag=f"b2_{j}")
                nc.sync.dma_start(tt[:], b2_d[o:o + s, :])
                b2_sb.append(tt)

            # sorted z^T resident in SBUF; column-chunked DMA
            zt_sb = []
            for i, (o, s) in enumerate(HID_CH):
                zt_sb.append(zpool.tile([s, BPC], F32R, tag=f"zt{i}",
                                        name=f"zt{i}"))
            nchunks = (BPC + CHUNK - 1) // CHUNK
            for c in range(nchunks):
                c0 = c * CHUNK
                ncs = min(CHUNK, BPC - c0)
                for i, (o, s) in enumerate(HID_CH):
                    nc.sync.dma_start(zt_sb[i][:, c0:c0 + ncs],
                                      zt_d[o:o + s, c0:c0 + ncs])

            goff = 0
            for k in range(MAX_N - 1):
                mk = mlist[k]
                if mk == 0:
                    continue
                # W1k = diag(key_k) @ W1 on the vector engine (converts to f32r)
                w1k = []
                for i in range(len(HID_CH)):
                    t = kpool.tile([HID_CH[i][1], DEC_MID], F32R,
                                   tag=f"w1k{i}", name=f"w1k{i}")
                    nc.vector.tensor_scalar_mul(t[:], w1_sb[i][:],
                                                kt_sb[i][:, k:k + 1])
                    w1k.append(t)
                for c0 in range(0, mk, CHUNK):
                    ncs = min(CHUNK, mk - c0)
                    hts = []
                    for m, (mo, ms) in enumerate(DEC_CH):
                        ph = ps_h.tile([ms, ncs], F32, tag=f"ph{m}")
                        for i in range(len(HID_CH)):
                            nc.tensor.matmul(
                                ph[:], lhsT=w1k[i][:, mo:mo + ms],
                                rhs=zt_sb[i][:, c0:c0 + ncs],
                                start=(i == 0), stop=(i == len(HID_CH) - 1))
                        ht = hpool.tile([ms, ncs], F32R, tag=f"h{m}")
                        nc.scalar.activation(ht[:], ph[:], AF.Relu,
                                             bias=b1_sb[m][:])
                        hts.append(ht)
                    for j, (jo, js) in enumerate(DIM_CH):
                        px = ps_x.tile([js, ncs], F32, tag=f"px{j}")
                        for m in range(len(DEC_CH)):
                            nc.tensor.matmul(
                                px[:], lhsT=w2_sb[m][:, jo:jo + js],
                                rhs=hts[m][:],
                                start=(m == 0), stop=(m == len(DEC_CH) - 1))
                        xs = xpool.tile([js, ncs], F32, tag=f"xs{j}")
                        nc.vector.tensor_scalar_add(xs[:], px[:], b2_sb[j][:])
                        nc.sync.dma_start(
                            xt_d[jo:jo + js, goff + c0:goff + c0 + ncs], xs[:])
                goff += mk

    nc.compile()
    _compiled[key] = nc
    return nc


# ---------------------------------------------------------------- host math
def _keys_table(key_W1, key_b1, key_g1, key_be1, key_W2, key_b2):
    """The key_net on the 17 one-hot rows, in float64."""
    pre = key_W1.astype(np.float64) + key_b1.astype(np.float64)
    m = pre.mean(-1, keepdims=True)
    v = pre.var(-1, keepdims=True)
    ln = (pre - m) / np.sqrt(v + EPS) * key_g1.astype(np.float64) \
        + key_be1.astype(np.float64)
    keys = np.maximum(ln, 0.0) @ key_W2.astype(np.float64) \
        + key_b2.astype(np.float64)
    return keys  # [17, 512] float64


def _sizepred_exact(z_rows, sp_W1, sp_b1, sp_g1, sp_be1, sp_W2, sp_b2):
    """float64 replica of the reference size_pred MLP for selected rows."""
    h = z_rows.astype(np.float64) @ sp_W1.astype(np.float64) \
        + sp_b1.astype(np.float64)
    m = h.mean(-1, keepdims=True)
    v = h.var(-1, keepdims=True)
    ln = (h - m) / np.sqrt(v + EPS) * sp_g1.astype(np.float64) \
        + sp_be1.astype(np.float64)
    ln = np.maximum(ln, 0.0)
    return ln @ sp_W2.astype(np.float64) + sp_b2.astype(np.float64)


MARGIN = 0.05  # near-tie threshold for exact host recompute of argmax rows


def kernel(z, key_W1, key_b1, key_g1, key_be1, key_W2, key_b2,
           dec_W1, dec_b1, dec_W2, dec_b2,
           sp_W1, sp_b1, sp_g1, sp_be1, sp_W2, sp_b2):
    z = np.asarray(z, dtype=np.float32)
    to32 = lambda a: np.ascontiguousarray(np.asarray(a), dtype=np.float32)
    key_W1, key_b1, key_g1, key_be1, key_W2, key_b2 = map(
        to32, (key_W1, key_b1, key_g1, key_be1, key_W2, key_b2))
    dec_W1, dec_b1, dec_W2, dec_b2 = map(to32, (dec_W1, dec_b1, dec_W2, dec_b2))
    sp_W1, sp_b1, sp_g1, sp_be1, sp_W2, sp_b2 = map(
        to32, (sp_W1, sp_b1, sp_g1, sp_be1, sp_W2, sp_b2))

    col = lambda a: np.ascontiguousarray(a.reshape(-1, 1))

    # ---------------- launch 1: size_pred
    nc1 = _build_sizepred()
    zsh = z.reshape(NCORES, BPC, HID)
    in_maps = []
    for c in range(NCORES):
        zt = np.ascontiguousarray(zsh[c].T)  # [512, 4096]
        in_maps.append(dict(zt=zt, spw1=sp_W1, spb1=col(sp_b1),
                            spg1=col(sp_g1), spbe1=col(sp_be1),
                            spw2=sp_W2, spb2=col(sp_b2),
                            onesm=np.full((128, 1), 1.0 / SP_MID, np.float32),
                            ones1=np.ones((1, 128), np.float32)))
    res1 = bass_utils.run_bass_kernel_spmd(nc1, in_maps,
                                           core_ids=list(range(NCORES)))
    DIAG["res1"] = res1
    n_pred = np.concatenate(
        [res1.results[c]["npredT"].T for c in range(NCORES)], axis=0)
    DIAG["n_pred_dev"] = n_pred

    # ---------------- argmax with exact near-tie patch
    n = n_pred.argmax(-1).astype(np.int64)
    part = np.partition(n_pred, MAX_N - 2, axis=-1)
    margin = part[:, -1] - part[:, -2]
    risky = np.flatnonzero(margin < MARGIN)
    DIAG["n_risky"] = len(risky)
    if len(risky):
        np_exact = _sizepred_exact(z[risky], sp_W1, sp_b1, sp_g1, sp_be1,
                                   sp_W2, sp_b2)
        n[risky] = np_exact.argmax(-1)

    # ---------------- ragged structure
    T = int(n.sum())
    batch = np.repeat(np.arange(B, dtype=np.int64), n).astype(np.int32)
    starts = np.zeros(B, dtype=np.int64)
    np.cumsum(n[:-1], out=starts[1:])

    keys64 = _keys_table(key_W1, key_b1, key_g1, key_be1, key_W2, key_b2)
    keysT = np.ascontiguousarray(keys64.T.astype(np.float32))  # [512, 17]

    # Deal the globally size-sorted samples round-robin to cores: every core
    # sees a nearly identical descending-n profile, so per-k group sizes
    # (and therefore padding and the slowest core) are balanced.
    gorder = np.argsort(-n, kind="stable")            # [B] global desc order
    core_samples = [gorder[c::NCORES] for c in range(NCORES)]  # each desc
    mks = np.stack([(n[cs][:, None] > np.arange(MAX_N - 1)[None, :]).sum(0)
                    for cs in core_samples])          # [8, 16]
    # common padded sizes; even-padded (fp32r matmul dst width must be even)
    mlist = [min(BPC, int(m + (m & 1))) for m in mks.max(0).astype(int)]
    tpad = int(sum(mlist))

    # ---------------- launch 2: decoder
    nc2 = _build_decoder(mlist, tpad)
    in_maps = []
    for c in range(NCORES):
        zs = z[core_samples[c]]                       # sorted rows [4096, 512]
        zts = np.ascontiguousarray(zs.T)              # [512, 4096]
        in_maps.append(dict(zts=zts, dw1=dec_W1, db1=col(dec_b1),
                            dw2=dec_W2, db2=col(dec_b2), keysT=keysT))
    res2 = bass_utils.run_bass_kernel_spmd(nc2, in_maps,
                                           core_ids=list(range(NCORES)))
    DIAG["res2"] = res2

    # ---------------- host scatter back to ragged order
    offs = np.concatenate([[0], np.cumsum(mlist)]).astype(np.int64)
    x = np.empty((T, DIM), dtype=np.float32)
    for c in range(NCORES):
        xt = res2.results[c]["xT"]                    # [256, tpad]
        cs = core_samples[c]
        src_cols, dst_rows = [], []
        for k in range(MAX_N - 1):
            mk = int(mks[c, k])
            if mk == 0:
                continue
            dst_rows.append(starts[cs[:mk]] + k)
            src_cols.append(offs[k] + np.arange(mk))
        dst = np.concatenate(dst_rows)
        src = np.concatenate(src_cols)
        x[dst] = np.ascontiguousarray(xt.T)[src]
    return x, batch


# revision 22
# speedup vs baseline: 1.3584x; 1.0457x over previous
"""Trainium2 Bass kernel for nn_Decoder_12128987644664.

Pipeline (per input contract: takes FULL inputs, returns FULL output):
  1. Launch 1 (8 cores, data-parallel over batch): size_pred MLP
     (Linear -> LayerNorm -> ReLU -> Linear) on zT shards, emitting
     n_pred^T [17, 4096] per core.  Matmuls run in float32r.
  2. Host: argmax over n_pred (with exact float64 recompute of near-tie
     rows so the ragged sizes match a faithful fp32 reference bit-for-bit),
     ragged index construction, and a per-core sort of samples by
     descending n.
  3. Launch 2 (8 cores): the decoder MLP over the ragged rows.  Rows are
     grouped by within-sample position k; since
     (z_b * key_k) @ W1 == z_b @ (diag(key_k) @ W1), each group k is a
     dense matmul over a prefix of the sorted z shard (kept resident in
     SBUF), with the key folded into W1 on the scalar engine.  No gather,
     no HBM read amplification.
  4. Host: scatter rows back to the reference ragged order.

The key_net is a 17-row table (one-hot @ W is a row lookup) computed on
host in float64.
"""

import os
import numpy as np

import concourse.bass as bass
import concourse.bacc as bacc
import concourse.tile as tile
from concourse import mybir
from concourse import bass_utils

F32 = mybir.dt.float32
F32R = mybir.dt.float32r
AF = mybir.ActivationFunctionType

NCORES = 8
B = 32768
BPC = B // NCORES          # 4096 batch rows per core
HID = 512
DIM = 256
MAX_N = 17
SP_MID = (HID + MAX_N) // 2   # 264
DEC_MID = (HID + DIM) // 2    # 384
EPS = 1e-5
CHUNK = 512                # moving-dim chunk (fp32 moving max)

# partition chunking helpers: list of (offset, size) covering `total`
def _pchunks(total):
    out = []
    off = 0
    while off < total:
        sz = min(128, total - off)
        out.append((off, sz))
        off += sz
    return out

SP_CH = _pchunks(SP_MID)    # [(0,128),(128,128),(256,8)]
DEC_CH = _pchunks(DEC_MID)  # 3 x 128
DIM_CH = _pchunks(DIM)      # 2 x 128
HID_CH = _pchunks(HID)      # 4 x 128

_compiled = {}
DIAG = {}


# ---------------------------------------------------------------- launch 1
def _build_sizepred():
    key = ("sizepred", BPC)
    if key in _compiled:
        return _compiled[key]
    nc = bacc.Bacc("TRN2", target_bir_lowering=False, debug=False,
                   num_devices=NCORES)
    zt_d = nc.dram_tensor("zt", [HID, BPC], F32R, kind="ExternalInput").ap()
    w1_d = nc.dram_tensor("spw1", [HID, SP_MID], F32R, kind="ExternalInput").ap()
    b1_d = nc.dram_tensor("spb1", [SP_MID, 1], F32, kind="ExternalInput").ap()
    g1_d = nc.dram_tensor("spg1", [SP_MID, 1], F32, kind="ExternalInput").ap()
    be1_d = nc.dram_tensor("spbe1", [SP_MID, 1], F32, kind="ExternalInput").ap()
    w2_d = nc.dram_tensor("spw2", [SP_MID, MAX_N], F32R, kind="ExternalInput").ap()
    b2_d = nc.dram_tensor("spb2", [MAX_N, 1], F32, kind="ExternalInput").ap()
    onesm_d = nc.dram_tensor("onesm", [128, 1], F32R, kind="ExternalInput").ap()
    ones1_d = nc.dram_tensor("ones1", [1, 128], F32R, kind="ExternalInput").ap()
    np_d = nc.dram_tensor("npredT", [MAX_N, BPC], F32, kind="ExternalOutput").ap()

    with tile.TileContext(nc) as tc:
        with tc.tile_pool(name="const", bufs=1) as cpool, \
             tc.tile_pool(name="zts", bufs=1) as zpool, \
             tc.tile_pool(name="work", bufs=2) as wpool, \
             tc.tile_pool(name="small", bufs=2) as spool, \
             tc.tile_pool(name="ps_h", bufs=1, space="PSUM") as ps_h, \
             tc.tile_pool(name="ps_s", bufs=1, space="PSUM") as ps_s, \
             tc.tile_pool(name="ps_b", bufs=1, space="PSUM") as ps_b, \
             tc.tile_pool(name="ps_n", bufs=1, space="PSUM") as ps_n:

            # constants
            w1_sb = []
            for i, (o, s) in enumerate(HID_CH):
                t = cpool.tile([s, SP_MID], F32R, tag=f"w1_{i}")
                nc.sync.dma_start(t[:], w1_d[o:o + s, :])
                w1_sb.append(t)
            w2_sb, b1_sb, g1_sb, be1_sb = [], [], [], []
            for m, (o, s) in enumerate(SP_CH):
                t = cpool.tile([s, MAX_N], F32R, tag=f"w2_{m}")
                nc.sync.dma_start(t[:], w2_d[o:o + s, :])
                w2_sb.append(t)
                for lst, src, nm in ((b1_sb, b1_d, "b1"), (g1_sb, g1_d, "g1"),
                                     (be1_sb, be1_d, "be1")):
                    tt = cpool.tile([s, 1], F32, tag=f"{nm}_{m}")
                    nc.sync.dma_start(tt[:], src[o:o + s, :])
                    lst.append(tt)
            b2_sb = cpool.tile([MAX_N, 1], F32, tag="b2")
            nc.sync.dma_start(b2_sb[:], b2_d[:])
            # ones columns scaled by 1/SP_MID for mean via matmul
            onesm = cpool.tile([128, 1], F32R, tag="onesm")
            nc.sync.dma_start(onesm[:], onesm_d[:])
            # ones row for partition-broadcast (K=1 matmul)
            ones1 = cpool.tile([1, 128], F32R, tag="ones1")
            nc.sync.dma_start(ones1[:], ones1_d[:])
            epsb = cpool.tile([1, 1], F32, tag="epsb")
            nc.vector.memset(epsb[:], EPS)

            # z^T resident, DMA'd in column chunks
            zt_sb = []
            for i, (o, s) in enumerate(HID_CH):
                zt_sb.append(zpool.tile([s, BPC], F32R, tag=f"zt{i}",
                                        name=f"zt{i}"))
            nchunks = (BPC + CHUNK - 1) // CHUNK
            for c in range(nchunks):
                c0 = c * CHUNK
                ncs = min(CHUNK, BPC - c0)
                for i, (o, s) in enumerate(HID_CH):
                    nc.sync.dma_start(zt_sb[i][:, c0:c0 + ncs],
                                      zt_d[o:o + s, c0:c0 + ncs])

            def emit_a(c0, ncs):
                # mm1: hpre^T = spW1^T @ z^T + b1
                hpre = []
                for m, (mo, ms) in enumerate(SP_CH):
                    ph = ps_h.tile([ms, ncs], F32, tag=f"ph{m}",
                                   name=f"ph{m}")
                    for i, (io, isz) in enumerate(HID_CH):
                        nc.tensor.matmul(
                            ph[:], lhsT=w1_sb[i][:, mo:mo + ms],
                            rhs=zt_sb[i][:, c0:c0 + ncs],
                            start=(i == 0), stop=(i == len(HID_CH) - 1))
                    h = wpool.tile([ms, ncs], F32R, tag=f"hpre{m}",
                                   name=f"hpre{m}")
                    nc.scalar.activation(h[:], ph[:], AF.Identity,
                                         bias=b1_sb[m][:])
                    hpre.append(h)
                return hpre

            def emit_b(hpre, c0, ncs):
                # LN stats via ones-matmul (cross-partition sums)
                pmean = ps_s.tile([1, ncs], F32, tag="sum", name="pmean")
                for m, (mo, ms) in enumerate(SP_CH):
                    nc.tensor.matmul(pmean[:], lhsT=onesm[0:ms, :],
                                     rhs=hpre[m][:],
                                     start=(m == 0), stop=(m == len(SP_CH) - 1))
                hsq = []
                for m, (mo, ms) in enumerate(SP_CH):
                    t = wpool.tile([ms, ncs], F32R, tag=f"hsq{m}",
                                   name=f"hsq{m}")
                    nc.gpsimd.tensor_mul(t[:], hpre[m][:], hpre[m][:])
                    hsq.append(t)
                pmsq = ps_s.tile([1, ncs], F32, tag="ssq", name="pmsq")
                for m, (mo, ms) in enumerate(SP_CH):
                    nc.tensor.matmul(pmsq[:], lhsT=onesm[0:ms, :],
                                     rhs=hsq[m][:],
                                     start=(m == 0), stop=(m == len(SP_CH) - 1))
                # var = E[x^2] - mean^2 ; rstd = 1/sqrt(var+eps)
                msq = spool.tile([1, ncs], F32, tag="msq", name="msq")
                nc.scalar.activation(msq[:], pmean[:], AF.Square)
                var = spool.tile([1, ncs], F32, tag="var", name="var")
                nc.vector.tensor_sub(var[:], pmsq[:], msq[:])
                rstd_r = spool.tile([1, ncs], F32R, tag="rstd_r", name="rstd_r")
                nc.scalar.activation(rstd_r[:], var[:], AF.Abs_reciprocal_sqrt,
                                     bias=epsb[:])
                mrs_r = spool.tile([1, ncs], F32R, tag="mrs_r", name="mrs_r")
                nc.vector.tensor_mul(mrs_r[:], pmean[:], rstd_r[:])
                # broadcast across partitions via K=1 matmul
                br = ps_b.tile([128, ncs], F32, tag="br", name="br")
                nc.tensor.matmul(br[:], lhsT=ones1[:], rhs=rstd_r[:],
                                 start=True, stop=True)
                bm = ps_b.tile([128, ncs], F32, tag="bm", name="bm")
                nc.tensor.matmul(bm[:], lhsT=ones1[:], rhs=mrs_r[:],
                                 start=True, stop=True)
                # normalize + affine + relu: relu((h*br - bm)*g + be)
                nh = []
                for m, (mo, ms) in enumerate(SP_CH):
                    t1 = wpool.tile([ms, ncs], F32, tag=f"t1{m}",
                                    name=f"t1{m}")
                    nc.vector.tensor_mul(t1[:], hpre[m][:], br[0:ms, :])
                    t2 = wpool.tile([ms, ncs], F32, tag=f"t2{m}",
                                    name=f"t2{m}")
                    nc.vector.tensor_sub(t2[:], t1[:], bm[0:ms, :])
                    t3 = wpool.tile([ms, ncs], F32R, tag=f"nh{m}",
                                    name=f"nh{m}")
                    nc.scalar.activation(t3[:], t2[:], AF.Relu,
                                         bias=be1_sb[m][:], scale=g1_sb[m][:])
                    nh.append(t3)
                # mm2: n_pred^T = spW2^T @ nh + b2
                pnp = ps_n.tile([MAX_N, ncs], F32, tag="np", name="pnp")
                for m, (mo, ms) in enumerate(SP_CH):
                    nc.tensor.matmul(pnp[:], lhsT=w2_sb[m][:],
                                     rhs=nh[m][:],
                                     start=(m == 0), stop=(m == len(SP_CH) - 1))
                npo = wpool.tile([MAX_N, ncs], F32, tag="npo", name="npo")
                nc.scalar.activation(npo[:], pnp[:], AF.Identity,
                                     bias=b2_sb[:])
                nc.sync.dma_start(np_d[:, c0:c0 + ncs], npo[:])

            # Software-pipelined emission (phase A of chunk c+1 before
            # phase B of chunk c) keeps every engine fed across the deep
            # LN dependency chain.
            pending = None
            for c in range(nchunks):
                c0 = c * CHUNK
                ncs = min(CHUNK, BPC - c0)
                hpre = emit_a(c0, ncs)
                if pending is not None:
                    emit_b(*pending)
                pending = (hpre, c0, ncs)
            if pending is not None:
                emit_b(*pending)

    nc.compile()
    _compiled[key] = nc
    return nc


# ---------------------------------------------------------------- launch 2
def _build_decoder(mlist, tpad):
    key = ("decoder", tuple(mlist), tpad, BPC)
    if key in _compiled:
        return _compiled[key]
    nc = bacc.Bacc("TRN2", target_bir_lowering=False, debug=False,
                   num_devices=NCORES)
    zt_d = nc.dram_tensor("zts", [HID, BPC], F32R, kind="ExternalInput").ap()
    w1_d = nc.dram_tensor("dw1", [HID, DEC_MID], F32, kind="ExternalInput").ap()
    b1_d = nc.dram_tensor("db1", [DEC_MID, 1], F32, kind="ExternalInput").ap()
    w2_d = nc.dram_tensor("dw2", [DEC_MID, DIM], F32R, kind="ExternalInput").ap()
    b2_d = nc.dram_tensor("db2", [DIM, 1], F32, kind="ExternalInput").ap()
    kt_d = nc.dram_tensor("keysT", [HID, MAX_N], F32, kind="ExternalInput").ap()
    xt_d = nc.dram_tensor("xT", [DIM, tpad], F32, kind="ExternalOutput").ap()

    with tile.TileContext(nc) as tc:
        with tc.tile_pool(name="const", bufs=1) as cpool, \
             tc.tile_pool(name="zts", bufs=1) as zpool, \
             tc.tile_pool(name="w1k", bufs=2) as kpool, \
             tc.tile_pool(name="hber", bufs=3) as hpool, \
             tc.tile_pool(name="xout", bufs=2) as xpool, \
             tc.tile_pool(name="ps_h", bufs=2, space="PSUM") as ps_h, \
             tc.tile_pool(name="ps_x", bufs=1, space="PSUM") as ps_x:

            w1_sb, kt_sb = [], []
            for i, (o, s) in enumerate(HID_CH):
                t = cpool.tile([s, DEC_MID], F32, tag=f"w1_{i}")
                nc.sync.dma_start(t[:], w1_d[o:o + s, :])
                w1_sb.append(t)
                t = cpool.tile([s, MAX_N], F32, tag=f"kt_{i}")
                nc.sync.dma_start(t[:], kt_d[o:o + s, :])
                kt_sb.append(t)
            w2_sb, b1_sb = [], []
            for m, (o, s) in enumerate(DEC_CH):
                t = cpool.tile([s, DIM], F32R, tag=f"w2_{m}")
                nc.sync.dma_start(t[:], w2_d[o:o + s, :])
                w2_sb.append(t)
                tt = cpool.tile([s, 1], F32, tag=f"b1_{m}")
                nc.sync.dma_start(tt[:], b1_d[o:o + s, :])
                b1_sb.append(tt)
            b2_sb = []
            for j, (o, s) in enumerate(DIM_CH):
                tt = cpool.tile([s, 1], F32, tag=f"b2_{j}")
                nc.sync.dma_start(tt[:], b2_d[o:o + s, :])
                b2_sb.append(tt)

            # sorted z^T resident in SBUF; column-chunked DMA
            zt_sb = []
            for i, (o, s) in enumerate(HID_CH):
                zt_sb.append(zpool.tile([s, BPC], F32R, tag=f"zt{i}",
                                        name=f"zt{i}"))
            nchunks = (BPC + CHUNK - 1) // CHUNK
            for c in range(nchunks):
                c0 = c * CHUNK
                ncs = min(CHUNK, BPC - c0)
                for i, (o, s) in enumerate(HID_CH):
                    nc.sync.dma_start(zt_sb[i][:, c0:c0 + ncs],
                                      zt_d[o:o + s, c0:c0 + ncs])

            # Chunk list across all k-groups: (k, group_out_offset, c0, ncs).
            # Chunk sizes stay in [256, 512] and even: fp32r matmuls run at
            # quarter rate below 256 moving columns, and need even widths.
            def split_chunks(mk):
                sizes, rem = [], mk
                while rem >= 768:
                    sizes.append(CHUNK)
                    rem -= CHUNK
                if rem > CHUNK:
                    a = ((rem // 2) + 1) & ~1
                    sizes.extend([a, rem - a])
                elif rem > 0:
                    sizes.append(rem)
                return sizes

            chunks = []
            goff = 0
            for k in range(MAX_N - 1):
                mk = mlist[k]
                if mk == 0:
                    continue
                c0 = 0
                for ncs in split_chunks(mk):
                    chunks.append((k, goff, c0, ncs))
                    c0 += ncs
                goff += mk

            def emit_mm2(pend):
                hts, goff, c0, ncs = pend
                for j, (jo, js) in enumerate(DIM_CH):
                    px = ps_x.tile([js, ncs], F32, tag=f"px{j}",
                                   name=f"px{j}")
                    for m in range(len(DEC_CH)):
                        nc.tensor.matmul(
                            px[:], lhsT=w2_sb[m][:, jo:jo + js],
                            rhs=hts[m][:],
                            start=(m == 0), stop=(m == len(DEC_CH) - 1))
                    xs = xpool.tile([js, ncs], F32, tag=f"xs{j}",
                                    name=f"xs{j}")
                    nc.vector.tensor_scalar_add(xs[:], px[:], b2_sb[j][:])
                    nc.sync.dma_start(
                        xt_d[jo:jo + js, goff + c0:goff + c0 + ncs], xs[:])

            # Software-pipelined emission: mm1/relu of chunk c+1 is issued
            # before mm2 of chunk c, so the in-order PE queue never stalls
            # waiting for the scalar engine's relu.
            w1k, cur_k, pending = None, -1, None
            for (k, goff, c0, ncs) in chunks:
                if k != cur_k:
                    # W1k = diag(key_k) @ W1 on the vector engine (-> f32r)
                    w1k = []
                    for i in range(len(HID_CH)):
                        t = kpool.tile([HID_CH[i][1], DEC_MID], F32R,
                                       tag=f"w1k{i}", name=f"w1k{i}")
                        nc.vector.tensor_scalar_mul(t[:], w1_sb[i][:],
                                                    kt_sb[i][:, k:k + 1])
                        w1k.append(t)
                    cur_k = k
                hts = []
                for m, (mo, ms) in enumerate(DEC_CH):
                    ph = ps_h.tile([ms, ncs], F32, tag=f"ph{m}",
                                   name=f"ph{m}")
                    for i in range(len(HID_CH)):
                        nc.tensor.matmul(
                            ph[:], lhsT=w1k[i][:, mo:mo + ms],
                            rhs=zt_sb[i][:, c0:c0 + ncs],
                            start=(i == 0), stop=(i == len(HID_CH) - 1))
                    ht = hpool.tile([ms, ncs], F32R, tag=f"h{m}",
                                    name=f"h{m}")
                    nc.scalar.activation(ht[:], ph[:], AF.Relu,
                                         bias=b1_sb[m][:])
                    hts.append(ht)
                if pending is not None:
                    emit_mm2(pending)
                pending = (hts, goff, c0, ncs)
            if pending is not None:
                emit_mm2(pending)

    nc.compile()
    _compiled[key] = nc
    return nc


# ---------------------------------------------------------------- host math
def _keys_table(key_W1, key_b1, key_g1, key_be1, key_W2, key_b2):
    """The key_net on the 17 one-hot rows, in float64."""
    pre = key_W1.astype(np.float64) + key_b1.astype(np.float64)
    m = pre.mean(-1, keepdims=True)
    v = pre.var(-1, keepdims=True)
    ln = (pre - m) / np.sqrt(v + EPS) * key_g1.astype(np.float64) \
        + key_be1.astype(np.float64)
    keys = np.maximum(ln, 0.0) @ key_W2.astype(np.float64) \
        + key_b2.astype(np.float64)
    return keys  # [17, 512] float64


def _sizepred_exact(z_rows, sp_W1, sp_b1, sp_g1, sp_be1, sp_W2, sp_b2):
    """float64 replica of the reference size_pred MLP for selected rows."""
    h = z_rows.astype(np.float64) @ sp_W1.astype(np.float64) \
        + sp_b1.astype(np.float64)
    m = h.mean(-1, keepdims=True)
    v = h.var(-1, keepdims=True)
    ln = (h - m) / np.sqrt(v + EPS) * sp_g1.astype(np.float64) \
        + sp_be1.astype(np.float64)
    ln = np.maximum(ln, 0.0)
    return ln @ sp_W2.astype(np.float64) + sp_b2.astype(np.float64)


MARGIN = 0.05  # near-tie threshold for exact host recompute of argmax rows


def kernel(z, key_W1, key_b1, key_g1, key_be1, key_W2, key_b2,
           dec_W1, dec_b1, dec_W2, dec_b2,
           sp_W1, sp_b1, sp_g1, sp_be1, sp_W2, sp_b2):
    z = np.asarray(z, dtype=np.float32)
    to32 = lambda a: np.ascontiguousarray(np.asarray(a), dtype=np.float32)
    key_W1, key_b1, key_g1, key_be1, key_W2, key_b2 = map(
        to32, (key_W1, key_b1, key_g1, key_be1, key_W2, key_b2))
    dec_W1, dec_b1, dec_W2, dec_b2 = map(to32, (dec_W1, dec_b1, dec_W2, dec_b2))
    sp_W1, sp_b1, sp_g1, sp_be1, sp_W2, sp_b2 = map(
        to32, (sp_W1, sp_b1, sp_g1, sp_be1, sp_W2, sp_b2))

    col = lambda a: np.ascontiguousarray(a.reshape(-1, 1))

    # ---------------- launch 1: size_pred
    nc1 = _build_sizepred()
    zsh = z.reshape(NCORES, BPC, HID)
    in_maps = []
    for c in range(NCORES):
        zt = np.ascontiguousarray(zsh[c].T)  # [512, 4096]
        in_maps.append(dict(zt=zt, spw1=sp_W1, spb1=col(sp_b1),
                            spg1=col(sp_g1), spbe1=col(sp_be1),
                            spw2=sp_W2, spb2=col(sp_b2),
                            onesm=np.full((128, 1), 1.0 / SP_MID, np.float32),
                            ones1=np.ones((1, 128), np.float32)))
    res1 = bass_utils.run_bass_kernel_spmd(nc1, in_maps,
                                           core_ids=list(range(NCORES)))
    DIAG["res1"] = res1
    n_pred = np.concatenate(
        [res1.results[c]["npredT"].T for c in range(NCORES)], axis=0)
    DIAG["n_pred_dev"] = n_pred

    # ---------------- argmax with exact near-tie patch
    n = n_pred.argmax(-1).astype(np.int64)
    part = np.partition(n_pred, MAX_N - 2, axis=-1)
    margin = part[:, -1] - part[:, -2]
    risky = np.flatnonzero(margin < MARGIN)
    DIAG["n_risky"] = len(risky)
    if len(risky):
        np_exact = _sizepred_exact(z[risky], sp_W1, sp_b1, sp_g1, sp_be1,
                                   sp_W2, sp_b2)
        n[risky] = np_exact.argmax(-1)

    # ---------------- ragged structure
    T = int(n.sum())
    batch = np.repeat(np.arange(B, dtype=np.int64), n).astype(np.int32)
    starts = np.zeros(B, dtype=np.int64)
    np.cumsum(n[:-1], out=starts[1:])

    keys64 = _keys_table(key_W1, key_b1, key_g1, key_be1, key_W2, key_b2)
    keysT = np.ascontiguousarray(keys64.T.astype(np.float32))  # [512, 17]

    # Deal the globally size-sorted samples round-robin to cores: every core
    # sees a nearly identical descending-n profile, so per-k group sizes
    # (and therefore padding and the slowest core) are balanced.
    gorder = np.argsort(-n, kind="stable")            # [B] global desc order
    core_samples = [gorder[c::NCORES] for c in range(NCORES)]  # each desc
    mks = np.stack([(n[cs][:, None] > np.arange(MAX_N - 1)[None, :]).sum(0)
                    for cs in core_samples])          # [8, 16]
    # common padded sizes; even-padded (fp32r matmul dst width must be even)
    # and >=256 when nonzero (fp32r runs 4x slower below 256 moving columns)
    mlist = [0 if m == 0 else min(BPC, max(256, int(m + (m & 1))))
             for m in mks.max(0).astype(int)]
    tpad = int(sum(mlist))

    # ---------------- launch 2: decoder
    nc2 = _build_decoder(mlist, tpad)
    in_maps = []
    for c in range(NCORES):
        zs = z[core_samples[c]]                       # sorted rows [4096, 512]
        zts = np.ascontiguousarray(zs.T)              # [512, 4096]
        in_maps.append(dict(zts=zts, dw1=dec_W1, db1=col(dec_b1),
                            dw2=dec_W2, db2=col(dec_b2), keysT=keysT))
    res2 = bass_utils.run_bass_kernel_spmd(nc2, in_maps,
                                           core_ids=list(range(NCORES)))
    DIAG["res2"] = res2

    # ---------------- host scatter back to ragged order
    offs = np.concatenate([[0], np.cumsum(mlist)]).astype(np.int64)
    x = np.empty((T, DIM), dtype=np.float32)
    for c in range(NCORES):
        xt = res2.results[c]["xT"]                    # [256, tpad]
        cs = core_samples[c]
        src_cols, dst_rows = [], []
        for k in range(MAX_N - 1):
            mk = int(mks[c, k])
            if mk == 0:
                continue
            dst_rows.append(starts[cs[:mk]] + k)
            src_cols.append(offs[k] + np.arange(mk))
        dst = np.concatenate(dst_rows)
        src = np.concatenate(src_cols)
        x[dst] = np.ascontiguousarray(xt.T)[src]
    return x, batch


# revision 25
# speedup vs baseline: 1.3997x; 1.0304x over previous
"""Trainium2 Bass kernel for nn_Decoder_12128987644664.

Pipeline (per input contract: takes FULL inputs, returns FULL output):
  1. Launch 1 (8 cores, data-parallel over batch): size_pred MLP
     (Linear -> LayerNorm -> ReLU -> Linear) on zT shards, emitting
     n_pred^T [17, 4096] per core.  Matmuls run in float32r.
  2. Host: argmax over n_pred (with exact float64 recompute of near-tie
     rows so the ragged sizes match a faithful fp32 reference bit-for-bit),
     ragged index construction, and a per-core sort of samples by
     descending n.
  3. Launch 2 (8 cores): the decoder MLP over the ragged rows.  Rows are
     grouped by within-sample position k; since
     (z_b * key_k) @ W1 == z_b @ (diag(key_k) @ W1), each group k is a
     dense matmul over a prefix of the sorted z shard (kept resident in
     SBUF), with the key folded into W1 on the scalar engine.  No gather,
     no HBM read amplification.
  4. Host: scatter rows back to the reference ragged order.

The key_net is a 17-row table (one-hot @ W is a row lookup) computed on
host in float64.
"""

import os
import numpy as np

import concourse.bass as bass
import concourse.bacc as bacc
import concourse.tile as tile
from concourse import mybir
from concourse import bass_utils

F32 = mybir.dt.float32
F32R = mybir.dt.float32r
AF = mybir.ActivationFunctionType

NCORES = 8
B = 32768
BPC = B // NCORES          # 4096 batch rows per core
HID = 512
DIM = 256
MAX_N = 17
SP_MID = (HID + MAX_N) // 2   # 264
DEC_MID = (HID + DIM) // 2    # 384
EPS = 1e-5
CHUNK = 512                # moving-dim chunk (fp32 moving max)

# partition chunking helpers: list of (offset, size) covering `total`
def _pchunks(total):
    out = []
    off = 0
    while off < total:
        sz = min(128, total - off)
        out.append((off, sz))
        off += sz
    return out

SP_CH = _pchunks(SP_MID)    # [(0,128),(128,128),(256,8)]
DEC_CH = _pchunks(DEC_MID)  # 3 x 128
DIM_CH = _pchunks(DIM)      # 2 x 128
HID_CH = _pchunks(HID)      # 4 x 128

_compiled = {}
DIAG = {}


# ---------------------------------------------------------------- launch 1
def _build_sizepred():
    key = ("sizepred", BPC)
    if key in _compiled:
        return _compiled[key]
    nc = bacc.Bacc("TRN2", target_bir_lowering=False, debug=False,
                   num_devices=NCORES)
    zt_d = nc.dram_tensor("zt", [HID, BPC], F32R, kind="ExternalInput").ap()
    w1_d = nc.dram_tensor("spw1", [HID, SP_MID], F32R, kind="ExternalInput").ap()
    b1_d = nc.dram_tensor("spb1", [SP_MID, 1], F32, kind="ExternalInput").ap()
    g1_d = nc.dram_tensor("spg1", [SP_MID, 1], F32, kind="ExternalInput").ap()
    be1_d = nc.dram_tensor("spbe1", [SP_MID, 1], F32, kind="ExternalInput").ap()
    w2_d = nc.dram_tensor("spw2", [SP_MID, MAX_N], F32R, kind="ExternalInput").ap()
    b2_d = nc.dram_tensor("spb2", [MAX_N, 1], F32, kind="ExternalInput").ap()
    onesm_d = nc.dram_tensor("onesm", [128, 1], F32R, kind="ExternalInput").ap()
    ones1_d = nc.dram_tensor("ones1", [1, 128], F32R, kind="ExternalInput").ap()
    np_d = nc.dram_tensor("npredT", [MAX_N, BPC], F32, kind="ExternalOutput").ap()

    with tile.TileContext(nc) as tc:
        with tc.tile_pool(name="const", bufs=1) as cpool, \
             tc.tile_pool(name="zts", bufs=1) as zpool, \
             tc.tile_pool(name="work", bufs=2) as wpool, \
             tc.tile_pool(name="small", bufs=2) as spool, \
             tc.tile_pool(name="ps_h", bufs=1, space="PSUM") as ps_h, \
             tc.tile_pool(name="ps_s", bufs=1, space="PSUM") as ps_s, \
             tc.tile_pool(name="ps_b", bufs=1, space="PSUM") as ps_b, \
             tc.tile_pool(name="ps_n", bufs=1, space="PSUM") as ps_n:

            # constants; first-chunk z columns right after w1 so mm1 can start
            w1_sb = []
            for i, (o, s) in enumerate(HID_CH):
                t = cpool.tile([s, SP_MID], F32R, tag=f"w1_{i}")
                nc.sync.dma_start(t[:], w1_d[o:o + s, :])
                w1_sb.append(t)
            zt_sb = []
            for i, (o, s) in enumerate(HID_CH):
                zt_sb.append(zpool.tile([s, BPC], F32R, tag=f"zt{i}",
                                        name=f"zt{i}"))
            for i, (o, s) in enumerate(HID_CH):
                nc.sync.dma_start(zt_sb[i][:, 0:CHUNK], zt_d[o:o + s, 0:CHUNK])
            w2_sb, b1_sb, g1_sb, be1_sb = [], [], [], []
            for m, (o, s) in enumerate(SP_CH):
                t = cpool.tile([s, MAX_N], F32R, tag=f"w2_{m}")
                nc.sync.dma_start(t[:], w2_d[o:o + s, :])
                w2_sb.append(t)
                for lst, src, nm in ((b1_sb, b1_d, "b1"), (g1_sb, g1_d, "g1"),
                                     (be1_sb, be1_d, "be1")):
                    tt = cpool.tile([s, 1], F32, tag=f"{nm}_{m}")
                    nc.sync.dma_start(tt[:], src[o:o + s, :])
                    lst.append(tt)
            b2_sb = cpool.tile([MAX_N, 1], F32, tag="b2")
            nc.sync.dma_start(b2_sb[:], b2_d[:])
            # ones columns scaled by 1/SP_MID for mean via matmul
            onesm = cpool.tile([128, 1], F32R, tag="onesm")
            nc.sync.dma_start(onesm[:], onesm_d[:])
            # ones row for partition-broadcast (K=1 matmul)
            ones1 = cpool.tile([1, 128], F32R, tag="ones1")
            nc.sync.dma_start(ones1[:], ones1_d[:])
            epsb = cpool.tile([1, 1], F32, tag="epsb")
            nc.vector.memset(epsb[:], EPS)

            # remaining z^T column chunks
            nchunks = (BPC + CHUNK - 1) // CHUNK
            for c in range(1, nchunks):
                c0 = c * CHUNK
                ncs = min(CHUNK, BPC - c0)
                for i, (o, s) in enumerate(HID_CH):
                    nc.sync.dma_start(zt_sb[i][:, c0:c0 + ncs],
                                      zt_d[o:o + s, c0:c0 + ncs])

            def emit_a(c0, ncs):
                # mm1: hpre^T = spW1^T @ z^T + b1
                hpre = []
                for m, (mo, ms) in enumerate(SP_CH):
                    ph = ps_h.tile([ms, ncs], F32, tag=f"ph{m}",
                                   name=f"ph{m}")
                    for i, (io, isz) in enumerate(HID_CH):
                        nc.tensor.matmul(
                            ph[:], lhsT=w1_sb[i][:, mo:mo + ms],
                            rhs=zt_sb[i][:, c0:c0 + ncs],
                            start=(i == 0), stop=(i == len(HID_CH) - 1))
                    h = wpool.tile([ms, ncs], F32R, tag=f"hpre{m}",
                                   name=f"hpre{m}")
                    nc.scalar.activation(h[:], ph[:], AF.Identity,
                                         bias=b1_sb[m][:])
                    hpre.append(h)
                return hpre

            def emit_b(hpre, c0, ncs):
                # LN stats via ones-matmul (cross-partition sums)
                pmean = ps_s.tile([1, ncs], F32, tag="sum", name="pmean")
                for m, (mo, ms) in enumerate(SP_CH):
                    nc.tensor.matmul(pmean[:], lhsT=onesm[0:ms, :],
                                     rhs=hpre[m][:],
                                     start=(m == 0), stop=(m == len(SP_CH) - 1))
                hsq = []
                for m, (mo, ms) in enumerate(SP_CH):
                    t = wpool.tile([ms, ncs], F32R, tag=f"hsq{m}",
                                   name=f"hsq{m}")
                    nc.gpsimd.tensor_mul(t[:], hpre[m][:], hpre[m][:])
                    hsq.append(t)
                pmsq = ps_s.tile([1, ncs], F32, tag="ssq", name="pmsq")
                for m, (mo, ms) in enumerate(SP_CH):
                    nc.tensor.matmul(pmsq[:], lhsT=onesm[0:ms, :],
                                     rhs=hsq[m][:],
                                     start=(m == 0), stop=(m == len(SP_CH) - 1))
                # var = E[x^2] - mean^2 ; rstd = 1/sqrt(var+eps)
                msq = spool.tile([1, ncs], F32, tag="msq", name="msq")
                nc.scalar.activation(msq[:], pmean[:], AF.Square)
                var = spool.tile([1, ncs], F32, tag="var", name="var")
                nc.vector.tensor_sub(var[:], pmsq[:], msq[:])
                rstd_r = spool.tile([1, ncs], F32R, tag="rstd_r", name="rstd_r")
                nc.scalar.activation(rstd_r[:], var[:], AF.Abs_reciprocal_sqrt,
                                     bias=epsb[:])
                mrs_r = spool.tile([1, ncs], F32R, tag="mrs_r", name="mrs_r")
                nc.vector.tensor_mul(mrs_r[:], pmean[:], rstd_r[:])
                # broadcast across partitions via K=1 matmul
                br = ps_b.tile([128, ncs], F32, tag="br", name="br")
                nc.tensor.matmul(br[:], lhsT=ones1[:], rhs=rstd_r[:],
                                 start=True, stop=True)
                bm = ps_b.tile([128, ncs], F32, tag="bm", name="bm")
                nc.tensor.matmul(bm[:], lhsT=ones1[:], rhs=mrs_r[:],
                                 start=True, stop=True)
                # normalize + affine + relu: relu((h*br - bm)*g + be)
                nh = []
                for m, (mo, ms) in enumerate(SP_CH):
                    t1 = wpool.tile([ms, ncs], F32, tag=f"t1{m}",
                                    name=f"t1{m}")
                    nc.vector.tensor_mul(t1[:], hpre[m][:], br[0:ms, :])
                    t2 = wpool.tile([ms, ncs], F32, tag=f"t2{m}",
                                    name=f"t2{m}")
                    nc.vector.tensor_sub(t2[:], t1[:], bm[0:ms, :])
                    t3 = wpool.tile([ms, ncs], F32R, tag=f"nh{m}",
                                    name=f"nh{m}")
                    nc.scalar.activation(t3[:], t2[:], AF.Relu,
                                         bias=be1_sb[m][:], scale=g1_sb[m][:])
                    nh.append(t3)
                # mm2: n_pred^T = spW2^T @ nh + b2
                pnp = ps_n.tile([MAX_N, ncs], F32, tag="np", name="pnp")
                for m, (mo, ms) in enumerate(SP_CH):
                    nc.tensor.matmul(pnp[:], lhsT=w2_sb[m][:],
                                     rhs=nh[m][:],
                                     start=(m == 0), stop=(m == len(SP_CH) - 1))
                npo = wpool.tile([MAX_N, ncs], F32, tag="npo", name="npo")
                nc.scalar.activation(npo[:], pnp[:], AF.Identity,
                                     bias=b2_sb[:])
                nc.sync.dma_start(np_d[:, c0:c0 + ncs], npo[:])

            # Software-pipelined emission (phase A of chunk c+1 before
            # phase B of chunk c) keeps every engine fed across the deep
            # LN dependency chain.
            pending = None
            for c in range(nchunks):
                c0 = c * CHUNK
                ncs = min(CHUNK, BPC - c0)
                hpre = emit_a(c0, ncs)
                if pending is not None:
                    emit_b(*pending)
                pending = (hpre, c0, ncs)
            if pending is not None:
                emit_b(*pending)

    nc.compile()
    _compiled[key] = nc
    return nc


# ---------------------------------------------------------------- launch 2
def _build_decoder(mlist, tpad):
    key = ("decoder", tuple(mlist), tpad, BPC)
    if key in _compiled:
        return _compiled[key]
    nc = bacc.Bacc("TRN2", target_bir_lowering=False, debug=False,
                   num_devices=NCORES)
    zt_d = nc.dram_tensor("zts", [HID, BPC], F32R, kind="ExternalInput").ap()
    w1_d = nc.dram_tensor("dw1", [HID, DEC_MID], F32, kind="ExternalInput").ap()
    b1_d = nc.dram_tensor("db1", [DEC_MID, 1], F32, kind="ExternalInput").ap()
    w2_d = nc.dram_tensor("dw2", [DEC_MID, DIM], F32R, kind="ExternalInput").ap()
    b2_d = nc.dram_tensor("db2", [DIM, 1], F32, kind="ExternalInput").ap()
    kt_d = nc.dram_tensor("keysT", [HID, MAX_N], F32, kind="ExternalInput").ap()
    xt_d = nc.dram_tensor("xT", [DIM, tpad], F32, kind="ExternalOutput").ap()

    with tile.TileContext(nc) as tc:
        with tc.tile_pool(name="const", bufs=1) as cpool, \
             tc.tile_pool(name="zts", bufs=1) as zpool, \
             tc.tile_pool(name="w1k", bufs=2) as kpool, \
             tc.tile_pool(name="hber", bufs=3) as hpool, \
             tc.tile_pool(name="xout", bufs=2) as xpool, \
             tc.tile_pool(name="ps_h", bufs=2, space="PSUM") as ps_h, \
             tc.tile_pool(name="ps_x", bufs=1, space="PSUM") as ps_x:

            # First the tensors the first chunk needs: w1/kt (for W1k), the
            # first z column chunk, and b1; bulk z and mm2 consts after.
            w1_sb, kt_sb = [], []
            for i, (o, s) in enumerate(HID_CH):
                t = cpool.tile([s, DEC_MID], F32, tag=f"w1_{i}")
                nc.sync.dma_start(t[:], w1_d[o:o + s, :])
                w1_sb.append(t)
                t = cpool.tile([s, MAX_N], F32, tag=f"kt_{i}")
                nc.sync.dma_start(t[:], kt_d[o:o + s, :])
                kt_sb.append(t)
            zt_sb = []
            for i, (o, s) in enumerate(HID_CH):
                zt_sb.append(zpool.tile([s, BPC], F32R, tag=f"zt{i}",
                                        name=f"zt{i}"))
            nchunks = (BPC + CHUNK - 1) // CHUNK
            for i, (o, s) in enumerate(HID_CH):
                nc.sync.dma_start(zt_sb[i][:, 0:CHUNK], zt_d[o:o + s, 0:CHUNK])
            b1_sb = []
            for m, (o, s) in enumerate(DEC_CH):
                tt = cpool.tile([s, 1], F32, tag=f"b1_{m}")
                nc.sync.dma_start(tt[:], b1_d[o:o + s, :])
                b1_sb.append(tt)
            w2_sb = []
            for m, (o, s) in enumerate(DEC_CH):
                t = cpool.tile([s, DIM], F32R, tag=f"w2_{m}")
                nc.sync.dma_start(t[:], w2_d[o:o + s, :])
                w2_sb.append(t)
            b2_sb = []
            for j, (o, s) in enumerate(DIM_CH):
                tt = cpool.tile([s, 1], F32, tag=f"b2_{j}")
                nc.sync.dma_start(tt[:], b2_d[o:o + s, :])
                b2_sb.append(tt)
            for c in range(1, nchunks):
                c0 = c * CHUNK
                ncs = min(CHUNK, BPC - c0)
                for i, (o, s) in enumerate(HID_CH):
                    nc.sync.dma_start(zt_sb[i][:, c0:c0 + ncs],
                                      zt_d[o:o + s, c0:c0 + ncs])

            # Chunk list across all k-groups: (k, group_out_offset, c0, ncs).
            # Chunk sizes stay in [256, 512] and even: fp32r matmuls run at
            # quarter rate below 256 moving columns, and need even widths.
            def split_chunks(mk):
                sizes, rem = [], mk
                while rem >= 768:
                    sizes.append(CHUNK)
                    rem -= CHUNK
                if rem > CHUNK:
                    a = ((rem // 2) + 1) & ~1
                    sizes.extend([a, rem - a])
                elif rem > 0:
                    sizes.append(rem)
                return sizes

            chunks = []
            goff = 0
            for k in range(MAX_N - 1):
                mk = mlist[k]
                if mk == 0:
                    continue
                c0 = 0
                for ncs in split_chunks(mk):
                    chunks.append((k, goff, c0, ncs))
                    c0 += ncs
                goff += mk

            def emit_mm2(pend):
                hts, goff, c0, ncs = pend
                for j, (jo, js) in enumerate(DIM_CH):
                    px = ps_x.tile([js, ncs], F32, tag=f"px{j}",
                                   name=f"px{j}")
                    for m in range(len(DEC_CH)):
                        nc.tensor.matmul(
                            px[:], lhsT=w2_sb[m][:, jo:jo + js],
                            rhs=hts[m][:],
                            start=(m == 0), stop=(m == len(DEC_CH) - 1))
                    xs = xpool.tile([js, ncs], F32, tag=f"xs{j}",
                                    name=f"xs{j}")
                    nc.vector.tensor_scalar_add(xs[:], px[:], b2_sb[j][:])
                    nc.sync.dma_start(
                        xt_d[jo:jo + js, goff + c0:goff + c0 + ncs], xs[:])

            # Software-pipelined emission: mm1/relu of chunk c+1 is issued
            # before mm2 of chunk c, so the in-order PE queue never stalls
            # waiting for the scalar engine's relu.
            w1k, cur_k, pending = None, -1, None
            for (k, goff, c0, ncs) in chunks:
                if k != cur_k:
                    # W1k = diag(key_k) @ W1 on the vector engine (-> f32r)
                    w1k = []
                    for i in range(len(HID_CH)):
                        t = kpool.tile([HID_CH[i][1], DEC_MID], F32R,
                                       tag=f"w1k{i}", name=f"w1k{i}")
                        nc.vector.tensor_scalar_mul(t[:], w1_sb[i][:],
                                                    kt_sb[i][:, k:k + 1])
                        w1k.append(t)
                    cur_k = k
                hts = []
                for m, (mo, ms) in enumerate(DEC_CH):
                    ph = ps_h.tile([ms, ncs], F32, tag=f"ph{m}",
                                   name=f"ph{m}")
                    for i in range(len(HID_CH)):
                        nc.tensor.matmul(
                            ph[:], lhsT=w1k[i][:, mo:mo + ms],
                            rhs=zt_sb[i][:, c0:c0 + ncs],
                            start=(i == 0), stop=(i == len(HID_CH) - 1))
                    ht = hpool.tile([ms, ncs], F32R, tag=f"h{m}",
                                    name=f"h{m}")
                    nc.scalar.activation(ht[:], ph[:], AF.Relu,
                                         bias=b1_sb[m][:])
                    hts.append(ht)
                if pending is not None:
                    emit_mm2(pending)
                pending = (hts, goff, c0, ncs)
            if pending is not None:
                emit_mm2(pending)

    nc.compile()
    _compiled[key] = nc
    return nc


# ---------------------------------------------------------------- host math
def _keys_table(key_W1, key_b1, key_g1, key_be1, key_W2, key_b2):
    """The key_net on the 17 one-hot rows, in float64."""
    pre = key_W1.astype(np.float64) + key_b1.astype(np.float64)
    m = pre.mean(-1, keepdims=True)
    v = pre.var(-1, keepdims=True)
    ln = (pre - m) / np.sqrt(v + EPS) * key_g1.astype(np.float64) \
        + key_be1.astype(np.float64)
    keys = np.maximum(ln, 0.0) @ key_W2.astype(np.float64) \
        + key_b2.astype(np.float64)
    return keys  # [17, 512] float64


def _sizepred_exact(z_rows, sp_W1, sp_b1, sp_g1, sp_be1, sp_W2, sp_b2):
    """float64 replica of the reference size_pred MLP for selected rows."""
    h = z_rows.astype(np.float64) @ sp_W1.astype(np.float64) \
        + sp_b1.astype(np.float64)
    m = h.mean(-1, keepdims=True)
    v = h.var(-1, keepdims=True)
    ln = (h - m) / np.sqrt(v + EPS) * sp_g1.astype(np.float64) \
        + sp_be1.astype(np.float64)
    ln = np.maximum(ln, 0.0)
    return ln @ sp_W2.astype(np.float64) + sp_b2.astype(np.float64)


MARGIN = 0.05  # near-tie threshold for exact host recompute of argmax rows


def kernel(z, key_W1, key_b1, key_g1, key_be1, key_W2, key_b2,
           dec_W1, dec_b1, dec_W2, dec_b2,
           sp_W1, sp_b1, sp_g1, sp_be1, sp_W2, sp_b2):
    z = np.asarray(z, dtype=np.float32)
    to32 = lambda a: np.ascontiguousarray(np.asarray(a), dtype=np.float32)
    key_W1, key_b1, key_g1, key_be1, key_W2, key_b2 = map(
        to32, (key_W1, key_b1, key_g1, key_be1, key_W2, key_b2))
    dec_W1, dec_b1, dec_W2, dec_b2 = map(to32, (dec_W1, dec_b1, dec_W2, dec_b2))
    sp_W1, sp_b1, sp_g1, sp_be1, sp_W2, sp_b2 = map(
        to32, (sp_W1, sp_b1, sp_g1, sp_be1, sp_W2, sp_b2))

    col = lambda a: np.ascontiguousarray(a.reshape(-1, 1))

    # ---------------- launch 1: size_pred
    nc1 = _build_sizepred()
    zsh = z.reshape(NCORES, BPC, HID)
    in_maps = []
    for c in range(NCORES):
        zt = np.ascontiguousarray(zsh[c].T)  # [512, 4096]
        in_maps.append(dict(zt=zt, spw1=sp_W1, spb1=col(sp_b1),
                            spg1=col(sp_g1), spbe1=col(sp_be1),
                            spw2=sp_W2, spb2=col(sp_b2),
                            onesm=np.full((128, 1), 1.0 / SP_MID, np.float32),
                            ones1=np.ones((1, 128), np.float32)))
    res1 = bass_utils.run_bass_kernel_spmd(nc1, in_maps,
                                           core_ids=list(range(NCORES)))
    DIAG["res1"] = res1
    n_pred = np.concatenate(
        [res1.results[c]["npredT"].T for c in range(NCORES)], axis=0)
    DIAG["n_pred_dev"] = n_pred

    # ---------------- argmax with exact near-tie patch
    n = n_pred.argmax(-1).astype(np.int64)
    part = np.partition(n_pred, MAX_N - 2, axis=-1)
    margin = part[:, -1] - part[:, -2]
    risky = np.flatnonzero(margin < MARGIN)
    DIAG["n_risky"] = len(risky)
    if len(risky):
        np_exact = _sizepred_exact(z[risky], sp_W1, sp_b1, sp_g1, sp_be1,
                                   sp_W2, sp_b2)
        n[risky] = np_exact.argmax(-1)

    # ---------------- ragged structure
    T = int(n.sum())
    batch = np.repeat(np.arange(B, dtype=np.int64), n).astype(np.int32)
    starts = np.zeros(B, dtype=np.int64)
    np.cumsum(n[:-1], out=starts[1:])

    keys64 = _keys_table(key_W1, key_b1, key_g1, key_be1, key_W2, key_b2)
    keysT = np.ascontiguousarray(keys64.T.astype(np.float32))  # [512, 17]

    # Deal the globally size-sorted samples round-robin to cores: every core
    # sees a nearly identical descending-n profile, so per-k group sizes
    # (and therefore padding and the slowest core) are balanced.
    gorder = np.argsort(-n, kind="stable")            # [B] global desc order
    core_samples = [gorder[c::NCORES] for c in range(NCORES)]  # each desc
    mks = np.stack([(n[cs][:, None] > np.arange(MAX_N - 1)[None, :]).sum(0)
                    for cs in core_samples])          # [8, 16]
    # common padded sizes; even-padded (fp32r matmul dst width must be even)
    # and >=256 when nonzero (fp32r runs 4x slower below 256 moving columns)
    mlist = [0 if m == 0 else min(BPC, max(256, int(m + (m & 1))))
             for m in mks.max(0).astype(int)]
    tpad = int(sum(mlist))

    # ---------------- launch 2: decoder
    nc2 = _build_decoder(mlist, tpad)
    in_maps = []
    for c in range(NCORES):
        zs = z[core_samples[c]]                       # sorted rows [4096, 512]
        zts = np.ascontiguousarray(zs.T)              # [512, 4096]
        in_maps.append(dict(zts=zts, dw1=dec_W1, db1=col(dec_b1),
                            dw2=dec_W2, db2=col(dec_b2), keysT=keysT))
    res2 = bass_utils.run_bass_kernel_spmd(nc2, in_maps,
                                           core_ids=list(range(NCORES)))
    DIAG["res2"] = res2

    # ---------------- host scatter back to ragged order
    offs = np.concatenate([[0], np.cumsum(mlist)]).astype(np.int64)
    x = np.empty((T, DIM), dtype=np.float32)
    for c in range(NCORES):
        xt = res2.results[c]["xT"]                    # [256, tpad]
        cs = core_samples[c]
        src_cols, dst_rows = [], []
        for k in range(MAX_N - 1):
            mk = int(mks[c, k])
            if mk == 0:
                continue
            dst_rows.append(starts[cs[:mk]] + k)
            src_cols.append(offs[k] + np.arange(mk))
        dst = np.concatenate(dst_rows)
        src = np.concatenate(src_cols)
        x[dst] = np.ascontiguousarray(xt.T)[src]
    return x, batch
